# revision 13
# baseline (speedup 1.0000x reference)
"""AttentionBlock kernel for 8 Trainium2 NeuronCores.

Reference computation (per batch b):
    Q = x[b] @ Wq + bq            [S, D]
    K = x[b] @ Wk + bk            [S, D]
    V = x[b] @ Wv + bv            [S, D]
    scores = Q @ K^T              [S, S]   (unscaled)
    attn = softmax(scores, -1)
    out[b] = attn @ V / sqrt(D)

Key algebraic restructuring: softmax is invariant to score terms constant
along the key axis, so
    softmax(Q K^T) = softmax(A' x^T),  A' = x_q (Wq Wk^T) + 1 (Wk bq)^T
(M = Wq Wk^T and c = Wk bq are weight-only transforms, computed on host).
This removes the K projection entirely: per core the Q+K projections of
the previous version (384 matmuls incl. a KT DRAM staging round-trip)
become one A' = x_q M projection (128 matmuls) with c folded in as the
ACT eviction bias. bv passes through softmax (attn rows sum to 1), so
V = x @ Wv unbiased and bv/32 is added to the final output.

Sharding: 8 cores = 4 batches x 2 query-halves. V (x @ Wv, full seq) is
duplicated within the pair - no communication. Each core's xt columns are
rotated on host so its query block is always columns [0, NQ): attention
is permutation-equivariant over keys (scores columns and V rows permute
together), so the output is unchanged and one h-agnostic kernel serves
all 8 cores.

Per-core layout: all f32r operands are pre-rounded on host (13 mantissa
bits, matching DVE f32->f32r rounding) and DMA'd straight into resident
f32r SBUF slabs - no on-device rounding passes. All matmuls use free-dim
512 so the fp32r LDWEIGHTS (~200ns, re-issued per matmul) hides under the
moving-operand stream.
  - xT [D, S] f32r resident slab (64KB/part): moving operand for A'
    (query columns), stationary tiles for both V-proj and scores.
  - A' [dk, q] f32r (32KB/part): stationary M-tiles x moving xT columns,
    ACT Identity evict with bias c.
  - V [s, dv] bf16 (32KB/part): stationary xT s-tiles x moving Wv.
  - scoresT [s-tile, q 512] = xT-tile^T @ A'-block in PSUM; Exp evict to
    eT bf16 (32KB/part). No max subtraction: max score ~69 stays inside
    f32/bf16 range and softmax is shift-invariant.
  - rowsum[1, q] via ones^T @ eT on the PE; reciprocal on DVE;
    PE-transposed to per-partition [128,1].
  - attn-output psum[q-tile, dv] = eT^T @ V accumulated over 16 s-tiles;
    j-groups of 3/3/1/1 (6 PSUM banks) so the tail after the final matmul
    is small; evicted with ACT scale=recip/32 and a DVE +bv/32 add.
"""
import sys
from contextlib import ExitStack

sys.path.insert(0, "/opt/trn_rl_repo")

import numpy as np

P = 128
D = 1024            # d_in = d_k = d_v
S = 2048            # kv sequence per core (full batch seq)
NQ = 1024           # query rows per core
B = 4
KT = D // P         # 8 contraction tiles
ST = S // P         # 16 s tiles
QH = 512            # free-dim chunk (fp32r moving-operand limit)
DVC = 512           # dv chunk width

_CACHE = {}


def _build():
    import concourse.bacc as bacc
    import concourse.mybir as mybir
    import concourse.tile as tile

    F32 = mybir.dt.float32
    F32R = mybir.dt.float32r
    BF16 = mybir.dt.bfloat16
    AF = mybir.ActivationFunctionType

    nc = bacc.Bacc("TRN2", target_bir_lowering=False, debug=False, num_devices=8)

    # inputs staged on host in SBUF layout [part, chunk, t, col] so every
    # chunk DMA is 128 descriptors of contiguous 16KB per partition
    xt_d = nc.dram_tensor("xt", [P, 2 + S // QH, KT, QH], F32R, kind="ExternalInput")
    m_d = nc.dram_tensor("m", [P, D // QH, KT, QH], F32R, kind="ExternalInput")
    wv_d = nc.dram_tensor("wv", [P, D // QH, KT, QH], F32R, kind="ExternalInput")
    ct_d = nc.dram_tensor("ct", [P, KT], F32, kind="ExternalInput")
    bvb_d = nc.dram_tensor("bvb", [P, D], mybir.dt.bfloat16, kind="ExternalInput")
    o_d = nc.dram_tensor("o", [NQ, D], F32, kind="ExternalOutput")

    with tile.TileContext(nc) as tc:
        with (
            tc.tile_pool(name="const", bufs=1) as constp,
            tc.tile_pool(name="xrp", bufs=1) as xrp,
            tc.tile_pool(name="ap", bufs=1) as ap_pool,
            tc.tile_pool(name="misc", bufs=1) as miscp,
            tc.tile_pool(name="outp", bufs=4) as outp,
        ):
            ct_sb = constp.tile([P, KT], F32)
            bvb_sb = constp.tile([P, D], BF16)
            ones_f = constp.tile([P, 1], F32)
            nc.vector.memset(ones_f[:], 32.0)
            ones_b = constp.tile([P, 1], BF16)
            nc.vector.tensor_copy(ones_b[:], ones_f[:])
            ident = constp.tile([1, 1], F32)
            nc.vector.memset(ident[:], 1.0)

            xk = xrp.tile([P, S // QH, KT, QH], F32R)  # keys, original order
            A = ap_pool.tile([P, KT, NQ], F32R)    # [dk%128, dk//128, q]

            xq_es = ExitStack()
            xqp = xq_es.enter_context(tc.tile_pool(name="xqp", bufs=1))
            xq = xqp.tile([P, NQ // QH, KT, QH], F32R)  # own query rows
            # query chunks first on the sync ring, then the key copy
            for c in range(NQ // QH):
                nc.sync.dma_start(xq[:, c], xt_d.ap()[:, c])
            for c in range(S // QH):
                nc.sync.dma_start(xk[:, c], xt_d.ap()[:, 2 + c])

            wvp_es = ExitStack()
            wvp = wvp_es.enter_context(tc.tile_pool(name="wvp", bufs=1))
            proj_es = ExitStack()
            mwp = proj_es.enter_context(tc.tile_pool(name="mw", bufs=1))
            ppp_es = ExitStack()
            ppp = ppp_es.enter_context(
                tc.tile_pool(name="pp", bufs=4, space="PSUM"))

            m_sb = mwp.tile([P, D // QH, KT, QH], F32R)
            wv_sb = wvp.tile([P, D // QH, KT, QH], F32R)
            # m is the startup-critical load: alone on the scalar ring (ct
            # first - 4KB - so the A evictions' bias never waits), while x
            # streams on the sync ring
            nc.scalar.dma_start(ct_sb[:], ct_d.ap())
            for c in range(D // QH):
                nc.scalar.dma_start(m_sb[:, c], m_d.ap()[:, c])

            # ---- A' = x_q @ M + c (ACT bias), 128 matmuls ----
            # dk-halves outer so PE consumption follows the m chunk arrival
            # order (m0 serves both q-chunks before m1 is needed)
            for mh in range(2):
              for qc in range(NQ // QH):
                for dk in range(mh * 4, mh * 4 + 4):
                    ps = ppp.tile([P, QH], F32, tag="pp", name="ps")
                    for t in range(KT):
                        nc.tensor.matmul(
                            ps[:],
                            m_sb[:, dk // 4, t, (dk % 4) * P:(dk % 4 + 1) * P],
                            xq[:, qc, t],
                            start=(t == 0), stop=(t == KT - 1),
                        )
                    nc.scalar.activation(
                        A[:, dk, qc * QH:(qc + 1) * QH], ps[:],
                        AF.Identity, bias=ct_sb[:, dk:dk + 1],
                    )
            # wv/bvb DMAs issued after the A matmuls: they execute on the
            # rings during phase A, but no A instruction can wait on them
            for c in range(D // QH):
                nc.scalar.dma_start(wv_sb[:, c], wv_d.ap()[:, c])
            nc.sync.dma_start(bvb_sb[:], bvb_d.ap())
            proj_es.close()                       # free M

            # ---- V for own query rows only (the pair's halves are
            # complementary in original coordinates); AllGather completes it
            vo_es = ExitStack()
            vop = vo_es.enter_context(
                tc.tile_pool(name="vop", bufs=1, side="right"))
            dram_es = ExitStack()
            dramp = dram_es.enter_context(
                tc.tile_pool(name="dram", bufs=1, space="DRAM"))
            Vown = vop.tile([P, ST // 2, D], BF16)  # [s%128, s//128, dv]
            v_half = dramp.tile([NQ, D], BF16)
            v_full = dramp.tile([S, D], BF16)
            for dv in range(D // DVC):
                for st in range(ST // 2):
                    ps = ppp.tile([P, DVC], F32, tag="pp", name="ps")
                    for t in range(KT):
                        nc.tensor.matmul(
                            ps[:],
                            xq[:, st // 4, t, (st % 4) * P:(st % 4 + 1) * P],
                            wv_sb[:, dv, t],
                            start=(t == 0), stop=(t == KT - 1),
                        )
                    nc.scalar.copy(Vown[:, st, dv * DVC:(dv + 1) * DVC], ps[:])
            wvp_es.close()                        # free Wv
            ppp_es.close()
            xq_es.close()                         # free query chunks
            v_half_r = v_half.rearrange("(t p) n -> p t n", p=P)
            nc.gpsimd.dma_start(v_half_r[:], Vown[:])
            nc.gpsimd.collective_compute(
                "AllGather",
                mybir.AluOpType.bypass,
                replica_groups=[[0, 1], [2, 3], [4, 5], [6, 7]],
                ins=[v_half.opt()],
                outs=[v_full.opt()],
            )
            vo_es.close()
            vp_es = ExitStack()
            vp = vp_es.enter_context(tc.tile_pool(name="vp", bufs=1, side="right"))
            V = vp.tile([P, ST, D], BF16)          # [s%128, s//128, dv]
            v_full_r = v_full.rearrange("(t p) n -> p t n", p=P)
            # readback overlaps the scores phase on the sync ring
            for t2 in range(2):
                nc.sync.dma_start(V[:, t2 * 8:(t2 + 1) * 8],
                                  v_full_r[:, t2 * 8:(t2 + 1) * 8])

            # ---- attention ----
            etp_es = ExitStack()
            etp = etp_es.enter_context(tc.tile_pool(name="etp", bufs=1, side="right"))
            eT = etp.tile([P, ST, NQ], BF16, tag="eT", name="eT")
            psr_es = ExitStack()
            psr = psr_es.enter_context(
                tc.tile_pool(name="psr", bufs=2, space="PSUM"))
            scr_es = ExitStack()
            pss = scr_es.enter_context(
                tc.tile_pool(name="pss", bufs=2, space="PSUM"))
            # scoresT[s-tile, q] = xT-tile^T @ A'-block, accumulated over
            # dk; rowsum as one contiguous bf16 block afterwards (mixing the
            # bf16 ones-matmuls into the f32r stream costs a PE mode switch
            # every 8 matmuls)
            for st in range(ST):
                for qh in range(NQ // QH):
                    ps = pss.tile([P, QH], F32, tag="ps", name="ps")
                    for t in range(KT):
                        nc.tensor.matmul(
                            ps[:],
                            xk[:, st // 4, t, (st % 4) * P:(st % 4 + 1) * P],
                            A[:, t, qh * QH:(qh + 1) * QH],
                            start=(t == 0), stop=(t == KT - 1),
                        )
                    nc.scalar.activation(
                        eT[:, st, qh * QH:(qh + 1) * QH], ps[:], AF.Exp)
            rec32s = []
            for qh in range(NQ // QH):
                prs = psr.tile([1, QH], F32, tag="prs", name="prs")
                for st in range(ST):
                    nc.tensor.matmul(
                        prs[:], ones_b[:], eT[:, st, qh * QH:(qh + 1) * QH],
                        start=(st == 0), stop=(st == ST - 1))
                rec32 = miscp.tile([1, QH], F32, tag=f"rec32{qh}", name="rec32")
                nc.vector.reciprocal(rec32[:], prs[:])
                rec32s.append(rec32)
            scr_es.close()

            # attn @ V in j-groups of 3/3/1/1 (6 PSUM banks max, small tail)
            with (
                tc.tile_pool(name="pso", bufs=1, space="PSUM") as pso,
                tc.tile_pool(name="pst", bufs=1, space="PSUM") as pst,
            ):
                rcs = []
                groups = [(0, 1), (2, 3), (4, 5), (6,), (7,)]
                for gi, js in enumerate(groups):
                    pos = [
                        pso.tile([P, DVC], F32, tag=f"po{u}", name="po")
                        for u in range(len(js) * (D // DVC))
                    ]
                    for ji, j in enumerate(js):
                        for dv in range(D // DVC):
                            for st in range(ST):
                                nc.tensor.matmul(
                                    pos[ji * (D // DVC) + dv][:],
                                    eT[:, st, j * P:(j + 1) * P],
                                    V[:, st, dv * DVC:(dv + 1) * DVC],
                                    start=(st == 0), stop=(st == ST - 1),
                                )
                    if gi == 0:
                        # emitted after a dense MM batch so the ACT->DVE->PE
                        # reciprocal/transpose chain hides under the matmuls
                        for j in range(NQ // P):
                            qh, jq = divmod(j, QH // P)
                            pt = pst.tile([P, 1], F32, tag="pt", name="pt")
                            nc.tensor.transpose(
                                pt[:], rec32s[qh][:, jq * P:(jq + 1) * P],
                                ident[:])
                            rc = miscp.tile([P, 1], F32, tag=f"rc{j}", name="rc")
                            # 1/sqrt(d_k) is folded into ones=32 upstream
                            nc.vector.tensor_copy(rc[:], pt[:])
                            rcs.append(rc)
                    for ji, j in enumerate(js):
                        for dv in range(D // DVC):
                            po = pos[ji * (D // DVC) + dv]
                            osb = outp.tile([P, DVC], F32, tag="osb", name="osb")
                            nc.scalar.activation(osb[:], po[:], AF.Copy,
                                                 scale=rcs[j][:])
                            nc.vector.tensor_tensor(
                                osb[:], osb[:],
                                bvb_sb[:, dv * DVC:(dv + 1) * DVC],
                                op=mybir.AluOpType.add,
                            )
                            nc.scalar.dma_start(
                                o_d.ap()[j * P:(j + 1) * P,
                                         dv * DVC:(dv + 1) * DVC],
                                osb[:],
                            )
            psr_es.close()
            etp_es.close()
            vp_es.close()
            dram_es.close()
    nc.compile()
    return nc


def _get_nc():
    if "nc" not in _CACHE:
        _CACHE["nc"] = _build()
    return _CACHE["nc"]


def _preround(a, bits=13):
    # round mantissa to `bits` explicit bits (round-to-nearest), matching
    # the DVE f32->f32r rounding so raw DMA into f32r tiles is faithful
    u = np.ascontiguousarray(a, dtype=np.float32).view(np.uint32)
    shift = 23 - bits
    add = np.uint32(1 << (shift - 1))
    u = ((u.astype(np.uint64) + add) >> shift << shift).astype(np.uint32)
    return np.ascontiguousarray(u.view(np.float32))


def _in_maps(x, Wq, bq, Wk, bk, Wv, bv):
    import ml_dtypes
    def _stage(w):
        # [D, N] -> [128, N//512, 8, 512]: per-partition contiguous chunks
        return np.ascontiguousarray(
            w.reshape(KT, P, -1, QH).transpose(1, 2, 0, 3))

    M = _stage(_preround(
        np.asarray(Wq, np.float64) @ np.asarray(Wk, np.float64).T))
    c = (np.asarray(Wk, np.float64) @ np.asarray(bq, np.float64)).astype(np.float32)
    ct = np.ascontiguousarray(np.reshape(c, (KT, P)).T, dtype=np.float32)
    wv = _stage(_preround(Wv))
    bvb = np.ascontiguousarray(
        np.tile(np.asarray(bv, np.float32) / 32.0, (P, 1)).astype(ml_dtypes.bfloat16))
    x = np.asarray(x, np.float32)
    xk_stage = [_stage(_preround(x[b].T)) for b in range(B)]
    maps = []
    for cidx in range(8):
        b, h = cidx // 2, cidx % 2
        # chunks 0-1: own query rows; chunks 2-5: full x, original order
        xq = _stage(_preround(x[b, h * NQ:(h + 1) * NQ].T))
        xt = np.ascontiguousarray(np.concatenate([xq, xk_stage[b]], axis=1))
        maps.append({"xt": xt, "m": M, "wv": wv, "ct": ct, "bvb": bvb})
    return maps


def _run(inputs, trace=False, tmpdir=None):
    import time

    from concourse.bass_utils import run_bass_kernel_spmd

    nc = _get_nc()
    maps = _in_maps(**inputs)
    last_err = None
    for attempt in range(3):
        try:
            res = run_bass_kernel_spmd(nc, maps, core_ids=list(range(8)),
                                       trace=trace, tmpdir=tmpdir)
            break
        except Exception as e:  # transient NRT device errors recover on retry
            last_err = e
            time.sleep(10)
    else:
        raise last_err
    out = np.empty((B, 2 * NQ, D), dtype=np.float32)
    for cidx in range(8):
        b, h = cidx // 2, cidx % 2
        out[b, h * NQ:(h + 1) * NQ, :] = res.results[cidx]["o"]
    return out, res


def kernel(**inputs):
    out, _ = _run(inputs, trace=False)
    return out


# revision 15
# speedup vs baseline: 1.1248x; 1.1248x over previous
"""AttentionBlock kernel for 8 Trainium2 NeuronCores.

Reference computation (per batch b):
    Q = x[b] @ Wq + bq            [S, D]
    K = x[b] @ Wk + bk            [S, D]
    V = x[b] @ Wv + bv            [S, D]
    scores = Q @ K^T              [S, S]   (unscaled)
    attn = softmax(scores, -1)
    out[b] = attn @ V / sqrt(D)

Key algebraic restructuring: softmax is invariant to score terms constant
along the key axis, so
    softmax(Q K^T) = softmax(A' x^T),  A' = x_q (Wq Wk^T) + 1 (Wk bq)^T
(M = Wq Wk^T and c = Wk bq are weight-only transforms, computed on host).
This removes the K projection entirely: per core the Q+K projections of
the previous version (384 matmuls incl. a KT DRAM staging round-trip)
become one A' = x_q M projection (128 matmuls) with c folded in as the
ACT eviction bias. bv passes through softmax (attn rows sum to 1), so
V = x @ Wv unbiased and bv/32 is added to the final output.

Sharding: 8 cores = 4 batches x 2 query-halves. V (x @ Wv, full seq) is
duplicated within the pair - no communication. Each core's xt columns are
rotated on host so its query block is always columns [0, NQ): attention
is permutation-equivariant over keys (scores columns and V rows permute
together), so the output is unchanged and one h-agnostic kernel serves
all 8 cores.

Per-core layout: all f32r operands are pre-rounded on host (13 mantissa
bits, matching DVE f32->f32r rounding) and DMA'd straight into resident
f32r SBUF slabs - no on-device rounding passes. All matmuls use free-dim
512 so the fp32r LDWEIGHTS (~200ns, re-issued per matmul) hides under the
moving-operand stream.
  - xT [D, S] f32r resident slab (64KB/part): moving operand for A'
    (query columns), stationary tiles for both V-proj and scores.
  - A' [dk, q] f32r (32KB/part): stationary M-tiles x moving xT columns,
    ACT Identity evict with bias c.
  - V [s, dv] bf16 (32KB/part): stationary xT s-tiles x moving Wv.
  - scoresT [s-tile, q 512] = xT-tile^T @ A'-block in PSUM; Exp evict to
    eT bf16 (32KB/part). No max subtraction: max score ~69 stays inside
    f32/bf16 range and softmax is shift-invariant.
  - rowsum[1, q] via ones^T @ eT on the PE; reciprocal on DVE;
    PE-transposed to per-partition [128,1].
  - attn-output psum[q-tile, dv] = eT^T @ V accumulated over 16 s-tiles;
    j-groups of 3/3/1/1 (6 PSUM banks) so the tail after the final matmul
    is small; evicted with ACT scale=recip/32 and a DVE +bv/32 add.
"""
import sys
from contextlib import ExitStack

sys.path.insert(0, "/opt/trn_rl_repo")

import numpy as np

P = 128
D = 1024            # d_in = d_k = d_v
S = 2048            # kv sequence per core (full batch seq)
NQ = 1024           # query rows per core
B = 4
KT = D // P         # 8 contraction tiles
ST = S // P         # 16 s tiles
QH = 512            # free-dim chunk (fp32r moving-operand limit)
DVC = 512           # dv chunk width

_CACHE = {}


def _build():
    import concourse.bacc as bacc
    import concourse.mybir as mybir
    import concourse.tile as tile

    F32 = mybir.dt.float32
    F32R = mybir.dt.float32r
    BF16 = mybir.dt.bfloat16
    AF = mybir.ActivationFunctionType

    nc = bacc.Bacc("TRN2", target_bir_lowering=False, debug=False, num_devices=8)

    # inputs staged on host in SBUF layout [part, chunk, t, col] so every
    # chunk DMA is 128 descriptors of contiguous 16KB per partition
    xt_d = nc.dram_tensor("xt", [P, 2 + S // QH, KT, QH], F32R, kind="ExternalInput")
    m_d = nc.dram_tensor("m", [P, D // QH, KT, QH], F32R, kind="ExternalInput")
    wv_d = nc.dram_tensor("wv", [P, D // QH, KT, QH], F32R, kind="ExternalInput")
    ct_d = nc.dram_tensor("ct", [P, KT], F32, kind="ExternalInput")
    bvb_d = nc.dram_tensor("bvb", [P, D], mybir.dt.bfloat16, kind="ExternalInput")
    o_d = nc.dram_tensor("o", [NQ, D], F32, kind="ExternalOutput")

    with tile.TileContext(nc) as tc:
        with (
            tc.tile_pool(name="const", bufs=1) as constp,
            tc.tile_pool(name="xrp", bufs=1) as xrp,
            tc.tile_pool(name="ap", bufs=1) as ap_pool,
            tc.tile_pool(name="misc", bufs=1) as miscp,
            tc.tile_pool(name="outp", bufs=4) as outp,
        ):
            ct_sb = constp.tile([P, KT], F32)
            bvb_sb = constp.tile([P, D], BF16)
            ones_f = constp.tile([P, 1], F32)
            nc.vector.memset(ones_f[:], 32.0)
            ones_b = constp.tile([P, 1], BF16)
            nc.vector.tensor_copy(ones_b[:], ones_f[:])
            ident = constp.tile([1, 1], F32)
            nc.vector.memset(ident[:], 1.0)

            xk = xrp.tile([P, S // QH, KT, QH], F32R)  # keys, original order
            A = ap_pool.tile([P, KT, NQ], F32R)    # [dk%128, dk//128, q]

            xq_es = ExitStack()
            xqp = xq_es.enter_context(tc.tile_pool(name="xqp", bufs=1))
            xq = xqp.tile([P, NQ // QH, KT, QH], F32R)  # own query rows
            # query chunks first on the sync ring, then the key copy
            for c in range(NQ // QH):
                nc.sync.dma_start(xq[:, c], xt_d.ap()[:, c])
            for c in range(S // QH):
                nc.sync.dma_start(xk[:, c], xt_d.ap()[:, 2 + c])

            wvp_es = ExitStack()
            wvp = wvp_es.enter_context(tc.tile_pool(name="wvp", bufs=1))
            ppp_es = ExitStack()
            ppp = ppp_es.enter_context(
                tc.tile_pool(name="pp", bufs=4, space="PSUM"))

            wv_sb = wvp.tile([P, D // QH, KT, QH], F32R)
            # wv is now the startup-critical load: alone (after ct) on the
            # scalar ring while x streams on the sync ring
            nc.scalar.dma_start(ct_sb[:], ct_d.ap())
            for c in range(D // QH):
                nc.scalar.dma_start(wv_sb[:, c], wv_d.ap()[:, c])

            # ---- V for own query rows only, FIRST, so the AllGather gets
            # the widest window (it absorbs cross-core launch skew) ----
            vo_es = ExitStack()
            vop = vo_es.enter_context(
                tc.tile_pool(name="vop", bufs=1, side="right"))
            dram_es = ExitStack()
            dramp = dram_es.enter_context(
                tc.tile_pool(name="dram", bufs=1, space="DRAM"))
            Vown = vop.tile([P, ST // 2, D], BF16)  # [s%128, s//128, dv]
            v_half = dramp.tile([NQ, D], BF16)
            v_full = dramp.tile([S, D], BF16)
            for dv in range(D // DVC):
                for st in range(ST // 2):
                    ps = ppp.tile([P, DVC], F32, tag="pp", name="ps")
                    for t in range(KT):
                        nc.tensor.matmul(
                            ps[:],
                            xq[:, st // 4, t, (st % 4) * P:(st % 4 + 1) * P],
                            wv_sb[:, dv, t],
                            start=(t == 0), stop=(t == KT - 1),
                        )
                    nc.scalar.copy(Vown[:, st, dv * DVC:(dv + 1) * DVC], ps[:])
            v_half_r = v_half.rearrange("(t p) n -> p t n", p=P)
            nc.gpsimd.dma_start(v_half_r[:], Vown[:])
            nc.gpsimd.collective_compute(
                "AllGather",
                mybir.AluOpType.bypass,
                replica_groups=[[0, 1], [2, 3], [4, 5], [6, 7]],
                ins=[v_half.opt()],
                outs=[v_full.opt()],
            )

            wvp_es.close()                        # free Wv before m lands
            vo_es.close()
            # m DMAs issued after the V matmuls so nothing in V waits on
            # them; they stream on the scalar ring during the V phase
            proj_es = ExitStack()
            mwp = proj_es.enter_context(tc.tile_pool(name="mw", bufs=1))
            m_sb = mwp.tile([P, D // QH, KT, QH], F32R)
            for c in range(D // QH):
                nc.scalar.dma_start(m_sb[:, c], m_d.ap()[:, c])
            nc.sync.dma_start(bvb_sb[:], bvb_d.ap())

            # ---- A' = x_q @ M + c (ACT bias), 128 matmuls ----
            # dk-halves outer so PE consumption follows the m chunk arrival
            # order (m0 serves both q-chunks before m1 is needed)
            for mh in range(2):
              for qc in range(NQ // QH):
                for dk in range(mh * 4, mh * 4 + 4):
                    ps = ppp.tile([P, QH], F32, tag="pp", name="ps")
                    for t in range(KT):
                        nc.tensor.matmul(
                            ps[:],
                            m_sb[:, dk // 4, t, (dk % 4) * P:(dk % 4 + 1) * P],
                            xq[:, qc, t],
                            start=(t == 0), stop=(t == KT - 1),
                        )
                    nc.scalar.activation(
                        A[:, dk, qc * QH:(qc + 1) * QH], ps[:],
                        AF.Identity, bias=ct_sb[:, dk:dk + 1],
                    )
            proj_es.close()                       # free M
            ppp_es.close()
            xq_es.close()                         # free query chunks
            vp_es = ExitStack()
            vp = vp_es.enter_context(tc.tile_pool(name="vp", bufs=1, side="right"))
            V = vp.tile([P, ST, D], BF16)          # [s%128, s//128, dv]
            v_full_r = v_full.rearrange("(t p) n -> p t n", p=P)
            # readback overlaps the scores phase on the sync ring
            for t2 in range(2):
                nc.sync.dma_start(V[:, t2 * 8:(t2 + 1) * 8],
                                  v_full_r[:, t2 * 8:(t2 + 1) * 8])

            # ---- attention ----
            etp_es = ExitStack()
            etp = etp_es.enter_context(tc.tile_pool(name="etp", bufs=1, side="right"))
            eT = etp.tile([P, ST, NQ], BF16, tag="eT", name="eT")
            psr_es = ExitStack()
            psr = psr_es.enter_context(
                tc.tile_pool(name="psr", bufs=2, space="PSUM"))
            scr_es = ExitStack()
            pss = scr_es.enter_context(
                tc.tile_pool(name="pss", bufs=2, space="PSUM"))
            # scoresT[s-tile, q] = xT-tile^T @ A'-block, accumulated over
            # dk; rowsum as one contiguous bf16 block afterwards (mixing the
            # bf16 ones-matmuls into the f32r stream costs a PE mode switch
            # every 8 matmuls)
            for st in range(ST):
                for qh in range(NQ // QH):
                    ps = pss.tile([P, QH], F32, tag="ps", name="ps")
                    for t in range(KT):
                        nc.tensor.matmul(
                            ps[:],
                            xk[:, st // 4, t, (st % 4) * P:(st % 4 + 1) * P],
                            A[:, t, qh * QH:(qh + 1) * QH],
                            start=(t == 0), stop=(t == KT - 1),
                        )
                    nc.scalar.activation(
                        eT[:, st, qh * QH:(qh + 1) * QH], ps[:], AF.Exp)
            rec32s = []
            for qh in range(NQ // QH):
                prs = psr.tile([1, QH], F32, tag="prs", name="prs")
                for st in range(ST):
                    nc.tensor.matmul(
                        prs[:], ones_b[:], eT[:, st, qh * QH:(qh + 1) * QH],
                        start=(st == 0), stop=(st == ST - 1))
                rec32 = miscp.tile([1, QH], F32, tag=f"rec32{qh}", name="rec32")
                nc.vector.reciprocal(rec32[:], prs[:])
                rec32s.append(rec32)
            scr_es.close()

            # attn @ V in j-groups of 3/3/1/1 (6 PSUM banks max, small tail)
            with (
                tc.tile_pool(name="pso", bufs=1, space="PSUM") as pso,
                tc.tile_pool(name="pst", bufs=1, space="PSUM") as pst,
            ):
                rcs = []
                groups = [(0, 1), (2, 3), (4, 5), (6,), (7,)]
                for gi, js in enumerate(groups):
                    pos = [
                        pso.tile([P, DVC], F32, tag=f"po{u}", name="po")
                        for u in range(len(js) * (D // DVC))
                    ]
                    for ji, j in enumerate(js):
                        for dv in range(D // DVC):
                            for st in range(ST):
                                nc.tensor.matmul(
                                    pos[ji * (D // DVC) + dv][:],
                                    eT[:, st, j * P:(j + 1) * P],
                                    V[:, st, dv * DVC:(dv + 1) * DVC],
                                    start=(st == 0), stop=(st == ST - 1),
                                )
                    if gi == 0:
                        # emitted after a dense MM batch so the ACT->DVE->PE
                        # reciprocal/transpose chain hides under the matmuls
                        for j in range(NQ // P):
                            qh, jq = divmod(j, QH // P)
                            pt = pst.tile([P, 1], F32, tag="pt", name="pt")
                            nc.tensor.transpose(
                                pt[:], rec32s[qh][:, jq * P:(jq + 1) * P],
                                ident[:])
                            rc = miscp.tile([P, 1], F32, tag=f"rc{j}", name="rc")
                            # 1/sqrt(d_k) is folded into ones=32 upstream
                            nc.vector.tensor_copy(rc[:], pt[:])
                            rcs.append(rc)
                    for ji, j in enumerate(js):
                        for dv in range(D // DVC):
                            po = pos[ji * (D // DVC) + dv]
                            osb = outp.tile([P, DVC], F32, tag="osb", name="osb")
                            nc.scalar.activation(osb[:], po[:], AF.Copy,
                                                 scale=rcs[j][:])
                            nc.vector.tensor_tensor(
                                osb[:], osb[:],
                                bvb_sb[:, dv * DVC:(dv + 1) * DVC],
                                op=mybir.AluOpType.add,
                            )
                            nc.scalar.dma_start(
                                o_d.ap()[j * P:(j + 1) * P,
                                         dv * DVC:(dv + 1) * DVC],
                                osb[:],
                            )
            psr_es.close()
            etp_es.close()
            vp_es.close()
            dram_es.close()
    nc.compile()
    return nc


def _get_nc():
    if "nc" not in _CACHE:
        _CACHE["nc"] = _build()
    return _CACHE["nc"]


def _preround(a, bits=13):
    # round mantissa to `bits` explicit bits (round-to-nearest), matching
    # the DVE f32->f32r rounding so raw DMA into f32r tiles is faithful
    u = np.ascontiguousarray(a, dtype=np.float32).view(np.uint32)
    shift = 23 - bits
    add = np.uint32(1 << (shift - 1))
    u = ((u.astype(np.uint64) + add) >> shift << shift).astype(np.uint32)
    return np.ascontiguousarray(u.view(np.float32))


def _in_maps(x, Wq, bq, Wk, bk, Wv, bv):
    import ml_dtypes
    def _stage(w):
        # [D, N] -> [128, N//512, 8, 512]: per-partition contiguous chunks
        return np.ascontiguousarray(
            w.reshape(KT, P, -1, QH).transpose(1, 2, 0, 3))

    M = _stage(_preround(
        np.asarray(Wq, np.float64) @ np.asarray(Wk, np.float64).T))
    c = (np.asarray(Wk, np.float64) @ np.asarray(bq, np.float64)).astype(np.float32)
    ct = np.ascontiguousarray(np.reshape(c, (KT, P)).T, dtype=np.float32)
    wv = _stage(_preround(Wv))
    bvb = np.ascontiguousarray(
        np.tile(np.asarray(bv, np.float32) / 32.0, (P, 1)).astype(ml_dtypes.bfloat16))
    x = np.asarray(x, np.float32)
    xk_stage = [_stage(_preround(x[b].T)) for b in range(B)]
    maps = []
    for cidx in range(8):
        b, h = cidx // 2, cidx % 2
        # chunks 0-1: own query rows; chunks 2-5: full x, original order
        xq = _stage(_preround(x[b, h * NQ:(h + 1) * NQ].T))
        xt = np.ascontiguousarray(np.concatenate([xq, xk_stage[b]], axis=1))
        maps.append({"xt": xt, "m": M, "wv": wv, "ct": ct, "bvb": bvb})
    return maps


def _run(inputs, trace=False, tmpdir=None):
    import time

    from concourse.bass_utils import run_bass_kernel_spmd

    nc = _get_nc()
    maps = _in_maps(**inputs)
    last_err = None
    for attempt in range(3):
        try:
            res = run_bass_kernel_spmd(nc, maps, core_ids=list(range(8)),
                                       trace=trace, tmpdir=tmpdir)
            break
        except Exception as e:  # transient NRT device errors recover on retry
            last_err = e
            time.sleep(10)
    else:
        raise last_err
    out = np.empty((B, 2 * NQ, D), dtype=np.float32)
    for cidx in range(8):
        b, h = cidx // 2, cidx % 2
        out[b, h * NQ:(h + 1) * NQ, :] = res.results[cidx]["o"]
    return out, res


def kernel(**inputs):
    out, _ = _run(inputs, trace=False)
    return out
